# revision 23
# baseline (speedup 1.0000x reference)
"""Multi-head self-attention (RoPE, causal) on 8 trn2 NeuronCores.

Sharding: core c -> batch b = c // 4, head group g = c % 4 (4 heads each).
Each core:
  - projects Q,K,V for its batch / its 4 heads (token-major), applies RoPE
    via a gathered cos/sin table, transposes Q,K to [dk, S] layout,
  - computes scores^T = K^T-chunk x Q (k on partitions), exp on ACT,
    then out^T[d, q] accumulated via V'-stationary matmuls where V' carries
    an appended ones-column so the softmax sums ride along as row 64;
    score groups are emitted one group ahead of the V-matmuls so the PE
    keeps the ACT exp stream fed,
  - normalizes with a fast reciprocal + bf16 hi/lo ones-broadcast matmuls,
  - AllGathers (4-core batch group, split per head-pair to overlap with
    compute) the head-sharded attention output, packed so a per-core index
    gather (wsel input) extracts this core's 512-token q-slice with an
    SPMD-uniform program; the output projection consumes the first AG's
    half early (two-pass accumulation) to shorten the serial tail.
Host only reshapes/transposes/casts inputs and concatenates outputs.
"""

import sys

for _p in ("/opt/trn_rl_repo",):
    if _p not in sys.path:
        sys.path.append(_p)

import numpy as np
import ml_dtypes

import concourse.bass as bass
import concourse.mybir as mybir
import concourse.tile as tile
from concourse import bacc
from concourse.bass import ds, ts
from concourse.bass_utils import run_bass_kernel_spmd
from concourse.masks import make_identity

BF16 = mybir.dt.bfloat16
F32 = mybir.dt.float32
I32 = mybir.dt.int32

B, S, D = 2, 2048, 1024
H, DK = 16, 64
THETA = 10000.0
MAXPOS = 2048
N_CORES = 8
GROUPS = 4          # head groups (cores) per batch
HPC = H // GROUPS   # heads per core = 4
QKV_COLS = 3 * HPC * DK        # 768 per-core projection width
QK_COLS = 2 * HPC * DK         # 512 (Q then K)
NSC = S // 128                 # 16 token chunks
NQC = S // 512                 # 4 q column-chunks
QSLICE = S // GROUPS           # 512 output tokens per core
KGRP = 4                       # k-chunks per exp/V-matmul group
NKG = NSC // KGRP
MUL = mybir.AluOpType.mult
ADD = mybir.AluOpType.add
SUB = mybir.AluOpType.subtract


def _build():
    nc = bacc.Bacc("TRN2", num_devices=N_CORES)

    xT = nc.dram_tensor("xT", [D, S], BF16, kind="ExternalInput")
    wqkvT = nc.dram_tensor("wqkvT", [D, QKV_COLS], BF16, kind="ExternalInput")
    woT = nc.dram_tensor("woT", [D, D], BF16, kind="ExternalInput")
    cstab = nc.dram_tensor("cstab", [MAXPOS, 2 * DK], F32, kind="ExternalInput")
    pos = nc.dram_tensor("pos", [S, 1], I32, kind="ExternalInput")
    tri = nc.dram_tensor("tri", [128, 128], BF16, kind="ExternalInput")
    wsel = nc.dram_tensor("wsel", [QSLICE, 1], I32, kind="ExternalInput")
    finT = nc.dram_tensor("finT", [D, QSLICE], F32, kind="ExternalOutput")

    with tile.TileContext(nc) as tc:
        with (
            tc.tile_pool(name="const", bufs=1) as constp,
            tc.tile_pool(name="wts", bufs=1) as wtsp,
            tc.tile_pool(name="seq", bufs=1) as seqp,
            tc.tile_pool(name="xtp", bufs=3) as xtp,
            tc.tile_pool(name="ropet", bufs=2) as ropet,
            tc.tile_pool(name="attp", bufs=1) as attp,
            tc.tile_pool(name="pbig", bufs=3, space="PSUM") as pbig,
            tc.tile_pool(name="psmall", bufs=2, space="PSUM") as psmall,
            tc.tile_pool(name="dram", bufs=1, space="DRAM") as dramp,
        ):
            # ---------------- constants + resident weights ----------------
            ident = constp.tile([128, 128], BF16)
            make_identity(nc, ident[:])
            tri_t = constp.tile([128, 128], BF16)
            nc.sync.dma_start(out=tri_t[:], in_=tri[:])
            ones33 = constp.tile([33, 64], BF16)
            nc.vector.memset(ones33[:], 0.0)
            nc.vector.memset(ones33[0:1, :], 1.0)
            nc.vector.memset(ones33[ds(32, 1), :], 1.0)

            wt = wtsp.tile([128, 8, QKV_COLS], BF16)       # [dchunk][768]
            for k in range(8):
                nc.sync.dma_start(out=wt[:, k, :], in_=wqkvT[ts(k, 128), :])
            wo = wtsp.tile([128, 8, D], BF16)
            for k in range(8):
                nc.sync.dma_start(out=wo[:, k, :], in_=woT[ts(k, 128), :])

            # persistent per-core tensors
            qt = seqp.tile([128, 2, S], BF16)   # Q^T  [pair, dk(2x64), q]
            kt = seqp.tile([128, 2, S], BF16)   # K^T
            vv = seqp.tile([128, NSC, HPC * (DK + 1)], BF16)   # V + ones col
            vv4 = vv[:].rearrange("p c (h e) -> p c h e", h=HPC)
            nc.vector.memset(vv4[:, :, :, DK:DK + 1], 1.0)
            outv = attp.tile([DK + 1, HPC, S], F32)  # vals + sums row
            attT = attp.tile([128, 2, S], BF16)
            rwo = attp.tile([128, 8, QSLICE], BF16)
            wsel_sb = attp.tile([128, 4], I32)
            sr2 = attp.tile([33, S], BF16)   # recip hi (row 0) / lo (row 32)
            nc.vector.memset(sr2[:, :], 0.0)

            # ------------- projection + RoPE + transposes (scoped x) -------
            with tc.tile_pool(name="xap", bufs=1) as xap:
                xa = xap.tile([128, 8, S], BF16)           # resident x^T
                for k in range(8):
                    nc.sync.dma_start(out=xa[:, k, :], in_=xT[ts(k, 128), :])
                cs = xap.tile([128, NSC, 2 * DK], F32)     # cos/sin gather
                for c in range(NSC):
                    pidx = xtp.tile([128, 1], I32, tag="pidx")
                    nc.sync.dma_start(out=pidx[:], in_=pos[ts(c, 128), :])
                    nc.gpsimd.indirect_dma_start(
                        out=cs[:, c, :],
                        out_offset=None,
                        in_=cstab[:],
                        in_offset=bass.IndirectOffsetOnAxis(
                            ap=pidx[:, 0:1], axis=0),
                    )

                for sc in range(NSC):
                    ps = pbig.tile([128, QKV_COLS], F32, space="PSUM",
                                   tag="big")
                    for k in range(8):
                        nc.tensor.matmul(
                            ps[:, 0:512], lhsT=xa[:, k, ts(sc, 128)],
                            rhs=wt[:, k, 0:512],
                            start=(k == 0), stop=(k == 7),
                        )
                    for k in range(8):
                        nc.tensor.matmul(
                            ps[:, 512:768], lhsT=xa[:, k, ts(sc, 128)],
                            rhs=wt[:, k, 512:768],
                            start=(k == 0), stop=(k == 7),
                        )

                    # RoPE over the Q,K halves (cols 0:512), 8 blocks of 64
                    def qk_ap(off):
                        a = ps[:]
                        return bass.AP(a.tensor, a.offset + off,
                                       [a.ap[0], [DK, 2 * HPC], [2, DK // 2]])

                    def cs_ap(off):
                        a = cs[:, sc, :]
                        return bass.AP(a.tensor, a.offset + off,
                                       [a.ap[0], [0, 2 * HPC], [2, DK // 2]])

                    t1 = ropet.tile([128, 2 * HPC, DK // 2], F32, tag="t1")
                    t2 = ropet.tile([128, 2 * HPC, DK // 2], F32, tag="t2")
                    t3 = ropet.tile([128, 2 * HPC, DK // 2], F32, tag="t3")
                    t4 = ropet.tile([128, 2 * HPC, DK // 2], F32, tag="t4")
                    roped = ropet.tile([128, QK_COLS], BF16, tag="roped")

                    def roped_ap(off):
                        a = roped[:]
                        return bass.AP(a.tensor, a.offset + off,
                                       [a.ap[0], [DK, 2 * HPC], [2, DK // 2]])

                    nc.vector.tensor_tensor(t1[:], qk_ap(0), cs_ap(0), MUL)
                    nc.vector.tensor_tensor(t2[:], qk_ap(1), cs_ap(DK), MUL)
                    nc.vector.tensor_tensor(roped_ap(0), t1[:], t2[:], SUB)
                    nc.vector.tensor_tensor(t3[:], qk_ap(0), cs_ap(DK), MUL)
                    nc.vector.tensor_tensor(t4[:], qk_ap(1), cs_ap(0), MUL)
                    nc.vector.tensor_tensor(roped_ap(1), t3[:], t4[:], ADD)

                    # V columns (ones col already set)
                    nc.vector.tensor_copy(
                        vv4[:, sc, :, 0:DK],
                        ps[:, 512:768].rearrange("p (h e) -> p h e", h=HPC),
                    )

                    # transpose the 4 roped q/k 128-col blocks -> qt/kt
                    # via the DMA XBAR (keeps the PE free)
                    for t in range(4):
                        dst = qt if t < 2 else kt
                        nc.sync.dma_start(
                            out=dst[:, t % 2, ts(sc, 128)],
                            in_=roped[:, ts(t, 128)], transpose=True)

            # ------------- attention (per head) + split AllGather ----------
            agin = [dramp.tile([4 * 128, QSLICE], BF16, name=f"agin{p}")
                    for p in range(2)]
            agout = [dramp.tile([4 * 512, QSLICE], BF16, name=f"agout{p}")
                     for p in range(2)]

            with tc.tile_pool(name="post", bufs=1) as postp:
                def emit_scores(h, kg, expp):
                    pr, hf = h // 2, (h % 2) * DK
                    etiles = []
                    for j in range(kg * KGRP, (kg + 1) * KGRP):
                        et = expp.tile([128, S], BF16, tag="exp")
                        # scores^T for k-chunk j: [128 k, q], q >= 128*j
                        for qh in range(2):
                            q0 = max(128 * j, 1024 * qh)
                            q1 = 1024 * (qh + 1)
                            if q0 >= q1:
                                continue
                            sp = pbig.tile([128, 1024], F32, space="PSUM",
                                           tag="big")
                            for qq in (1024 * qh, 1024 * qh + 512):
                                a, bnd = max(q0, qq), min(q1, qq + 512)
                                if a >= bnd:
                                    continue
                                nc.tensor.matmul(
                                    sp[:, ds(a - 1024 * qh, bnd - a)],
                                    lhsT=kt[ds(hf, DK), pr, ts(j, 128)],
                                    rhs=qt[ds(hf, DK), pr, ds(a, bnd - a)],
                                    start=True, stop=True,
                                )
                            nc.scalar.activation(
                                et[:, ds(q0, q1 - q0)],
                                sp[:, ds(q0 - 1024 * qh, q1 - q0)],
                                mybir.ActivationFunctionType.Exp,
                            )
                        # mask the diagonal block (q < k -> 0)
                        nc.vector.tensor_tensor(
                            et[:, ts(j, 128)], et[:, ts(j, 128)], tri_t[:],
                            MUL)
                        etiles.append((j, et))
                    return etiles

                def emit_vmms(h, kg, etiles):
                    for qc in range(kg, NQC):
                        part = psmall.tile([DK + 1, 512], F32, space="PSUM",
                                           tag="small")
                        for idx, (j, et) in enumerate(etiles):
                            a = max(512 * qc, 128 * j)
                            nc.tensor.matmul(
                                part[:, ds(a - 512 * qc, 512 * (qc + 1) - a)],
                                lhsT=vv[:, j, ds(h * (DK + 1), DK + 1)],
                                rhs=et[:, ds(a, 512 * (qc + 1) - a)],
                                start=(idx == 0), stop=(idx == KGRP - 1),
                            )
                        if kg == 0:
                            nc.vector.tensor_copy(
                                outv[:, h, ts(qc, 512)], part[:])
                        else:
                            nc.vector.tensor_tensor(
                                outv[:, h, ts(qc, 512)],
                                outv[:, h, ts(qc, 512)], part[:], ADD)

                def emit_normalize(h):
                    pr, hf = h // 2, (h % 2) * DK
                    srow = constp.tile([1, S], F32, tag="srow")
                    nc.vector.tensor_copy(srow[:], outv[ds(DK, 1), h, :])
                    nc.vector.reciprocal_approx_fast(srow[:], srow[:])
                    nc.vector.tensor_copy(sr2[0:1, :], srow[:])
                    nc.vector.tensor_tensor(
                        sr2[ds(32, 1), :], srow[:], sr2[0:1, :], SUB)
                    for qc in range(NQC):
                        rb = psmall.tile([DK, 512], F32, space="PSUM",
                                         tag="small")
                        nc.tensor.matmul(rb[:], lhsT=ones33[:],
                                         rhs=sr2[:, ts(qc, 512)],
                                         start=True, stop=True)
                        nc.vector.tensor_tensor(
                            attT[ds(hf, DK), pr, ts(qc, 512)],
                            outv[0:DK, h, ts(qc, 512)], rb[:], MUL)
                    if h % 2 == 1:     # pair complete -> stage + AllGather
                        p = h // 2
                        nc.sync.dma_start(
                            out=agin[p][:].rearrange("(d j) q -> d j q",
                                                     j=GROUPS),
                            in_=attT[:, p, :].rearrange("p (j q) -> p j q",
                                                        j=GROUPS),
                        )
                        nc.gpsimd.collective_compute(
                            "AllGather",
                            mybir.AluOpType.bypass,
                            ins=[agin[p][:]],
                            outs=[agout[p][:]],
                            replica_groups=[[0, 1, 2, 3], [4, 5, 6, 7]],
                        )

                with tc.tile_pool(name="expp", bufs=2 * KGRP + 1) as expp:
                    pending = None
                    for h in range(HPC):
                        for kg in range(NKG):
                            cur = emit_scores(h, kg, expp)
                            if pending is not None:
                                ph, pkg, pet = pending
                                emit_vmms(ph, pkg, pet)
                                if pkg == NKG - 1:
                                    emit_normalize(ph)
                            pending = (h, kg, cur)
                    ph, pkg, pet = pending
                    emit_vmms(ph, pkg, pet)
                    emit_normalize(ph)

                # ------------- output projection (q-slice), two passes -----
                for c in range(4):
                    nc.sync.dma_start(
                        out=wsel_sb[:, c:c + 1], in_=wsel[ts(c, 128), :])
                for dp in (0, 2, 4, 6, 1, 3, 5, 7):
                    nc.gpsimd.indirect_dma_start(
                        out=rwo[:, dp, :],
                        out_offset=None,
                        in_=agout[dp % 2][:],
                        in_offset=bass.IndirectOffsetOnAxis(
                            ap=wsel_sb[:, dp // 2:dp // 2 + 1], axis=0),
                    )
                fe = postp.tile([128, 8, QSLICE], F32)
                for ec in range(8):
                    fp = pbig.tile([128, QSLICE], F32, space="PSUM", tag="big")
                    for i, dp in enumerate((0, 2, 4, 6)):
                        nc.tensor.matmul(
                            fp[:], lhsT=wo[:, dp, ts(ec, 128)],
                            rhs=rwo[:, dp, :],
                            start=(i == 0), stop=(i == 3),
                        )
                    nc.vector.tensor_copy(fe[:, ec, :], fp[:])
                for ec in range(8):
                    fp = pbig.tile([128, QSLICE], F32, space="PSUM", tag="big")
                    for i, dp in enumerate((1, 3, 5, 7)):
                        nc.tensor.matmul(
                            fp[:], lhsT=wo[:, dp, ts(ec, 128)],
                            rhs=rwo[:, dp, :],
                            start=(i == 0), stop=(i == 3),
                        )
                    fin_sb = xtp.tile([128, QSLICE], F32, tag="fin")
                    nc.vector.tensor_tensor(fin_sb[:], fe[:, ec, :], fp[:],
                                            ADD)
                    nc.sync.dma_start(out=finT[ts(ec, 128), :], in_=fin_sb[:])

    nc.compile()
    return nc


def _host_prep(x, token_positions, W_qkv, W_o):
    bf16 = ml_dtypes.bfloat16
    xT = np.ascontiguousarray(np.transpose(x, (0, 2, 1))).astype(bf16)  # [B,D,S]

    # per-group W_qkv^T slices (Q rows pre-scaled by 1/sqrt(dk))
    wq = W_qkv[0 * D:1 * D] * np.float32(1.0 / np.sqrt(DK))
    wk = W_qkv[1 * D:2 * D]
    wv = W_qkv[2 * D:3 * D]
    wslices = []
    for g in range(GROUPS):
        rows = slice(g * HPC * DK, (g + 1) * HPC * DK)
        wsl = np.concatenate([wq[rows], wk[rows], wv[rows]], axis=0)  # [768, D]
        wslices.append(np.ascontiguousarray(wsl.T).astype(bf16))      # [D, 768]

    woT = np.ascontiguousarray(W_o.T).astype(bf16)                    # [D, D]

    idx = np.arange(DK // 2, dtype=np.float64)
    freqs = 1.0 / (THETA ** (2.0 * idx / DK))
    ang = np.arange(MAXPOS, dtype=np.float64)[:, None] * freqs[None, :]
    cstab = np.zeros((MAXPOS, 2 * DK), dtype=np.float32)
    cstab[:, 0:DK:2] = np.cos(ang)
    cstab[:, 1:DK:2] = np.cos(ang)
    cstab[:, DK::2] = np.sin(ang)
    cstab[:, DK + 1::2] = np.sin(ang)

    tri = (np.arange(128)[None, :] >= np.arange(128)[:, None]).astype(bf16)

    posi = np.asarray(token_positions).astype(np.int32).reshape(B, S, 1)

    # wsel[c*128 + r] selects row 512*c + 4*r + g of agout[dp % 2] for
    # d-chunk dp = 2*c + (0 or 1); same values for both halves.
    rr = np.arange(QSLICE)
    in_maps = []
    for c in range(N_CORES):
        b, g = c // GROUPS, c % GROUPS
        wsel = 512 * (rr // 128) + 4 * (rr % 128) + g
        in_maps.append({
            "xT": np.asarray(xT[b]),
            "wqkvT": wslices[g],
            "woT": woT,
            "cstab": cstab,
            "pos": np.ascontiguousarray(posi[b]),
            "tri": tri,
            "wsel": wsel.astype(np.int32).reshape(QSLICE, 1),
        })
    return in_maps


def _assemble(results):
    out = np.empty((B, S, D), dtype=np.float32)
    for b in range(B):
        fullT = np.concatenate(
            [results[b * GROUPS + g]["finT"] for g in range(GROUPS)], axis=1)
        out[b] = fullT.T
    return out


_NC_CACHE = {}


def run(inputs, trace=False, **kw):
    if "nc" not in _NC_CACHE:
        _NC_CACHE["nc"] = _build()
    nc = _NC_CACHE["nc"]
    in_maps = _host_prep(**inputs)
    res = run_bass_kernel_spmd(
        nc, in_maps, core_ids=list(range(N_CORES)), trace=trace, **kw)
    return _assemble(res.results), res


def kernel(**inputs):
    out, _ = run(inputs, trace=False)
    return out


# revision 24
# speedup vs baseline: 1.2545x; 1.2545x over previous
"""Multi-head self-attention (RoPE, causal) on 8 trn2 NeuronCores.

Sharding: core c -> batch b = c // 4, head group g = c % 4 (4 heads each).
Each core:
  - projects Q,K,V for its batch / its 4 heads (token-major), applies RoPE
    via a gathered cos/sin table, transposes Q,K to [dk, S] layout,
  - computes scores^T = K^T-chunk x Q (k on partitions), exp on ACT,
    then out^T[d, q] accumulated via V'-stationary matmuls where V' carries
    an appended ones-column so the softmax sums ride along as row 64;
    score groups are emitted one group ahead of the V-matmuls so the PE
    keeps the ACT exp stream fed,
  - normalizes with a fast reciprocal + bf16 hi/lo ones-broadcast matmuls,
  - AllGathers (4-core batch group, split per head-pair to overlap with
    compute) the head-sharded attention output, packed so a per-core index
    gather (wsel input) extracts this core's 512-token q-slice with an
    SPMD-uniform program; the output projection consumes the first AG's
    half early (two-pass accumulation) to shorten the serial tail.
Host only reshapes/transposes/casts inputs and concatenates outputs.
"""

import sys

for _p in ("/opt/trn_rl_repo",):
    if _p not in sys.path:
        sys.path.append(_p)

import numpy as np
import ml_dtypes

import concourse.bass as bass
import concourse.mybir as mybir
import concourse.tile as tile
from concourse import bacc
from concourse.bass import ds, ts
from concourse.bass_utils import run_bass_kernel_spmd
from concourse.masks import make_identity

BF16 = mybir.dt.bfloat16
F32 = mybir.dt.float32
I32 = mybir.dt.int32

B, S, D = 2, 2048, 1024
H, DK = 16, 64
THETA = 10000.0
MAXPOS = 2048
N_CORES = 8
GROUPS = 4          # head groups (cores) per batch
HPC = H // GROUPS   # heads per core = 4
QKV_COLS = 3 * HPC * DK        # 768 per-core projection width
QK_COLS = 2 * HPC * DK         # 512 (Q then K)
NSC = S // 128                 # 16 token chunks
NQC = S // 512                 # 4 q column-chunks
QSLICE = S // GROUPS           # 512 output tokens per core
KGRP = 4                       # k-chunks per exp/V-matmul group
NKG = NSC // KGRP
MUL = mybir.AluOpType.mult
ADD = mybir.AluOpType.add
SUB = mybir.AluOpType.subtract


def _build():
    nc = bacc.Bacc("TRN2", num_devices=N_CORES)

    xT = nc.dram_tensor("xT", [D, S], BF16, kind="ExternalInput")
    wqkvT = nc.dram_tensor("wqkvT", [D, QKV_COLS], BF16, kind="ExternalInput")
    woT = nc.dram_tensor("woT", [D, D], BF16, kind="ExternalInput")
    cstab = nc.dram_tensor("cstab", [MAXPOS, 2 * DK], F32, kind="ExternalInput")
    pos = nc.dram_tensor("pos", [S, 1], I32, kind="ExternalInput")
    tri = nc.dram_tensor("tri", [128, 128], BF16, kind="ExternalInput")
    wsel = nc.dram_tensor("wsel", [QSLICE, 1], I32, kind="ExternalInput")
    finT = nc.dram_tensor("finT", [D, QSLICE], F32, kind="ExternalOutput")

    with tile.TileContext(nc) as tc:
        with (
            tc.tile_pool(name="const", bufs=1) as constp,
            tc.tile_pool(name="wts", bufs=1) as wtsp,
            tc.tile_pool(name="seq", bufs=1) as seqp,
            tc.tile_pool(name="xtp", bufs=3) as xtp,
            tc.tile_pool(name="ropet", bufs=2) as ropet,
            tc.tile_pool(name="attp", bufs=1) as attp,
            tc.tile_pool(name="pbig", bufs=3, space="PSUM") as pbig,
            tc.tile_pool(name="psmall", bufs=2, space="PSUM") as psmall,
            tc.tile_pool(name="dram", bufs=1, space="DRAM") as dramp,
        ):
            # ---------------- constants + resident weights ----------------
            ident = constp.tile([128, 128], BF16)
            make_identity(nc, ident[:])
            tri_t = constp.tile([128, 128], BF16)
            nc.sync.dma_start(out=tri_t[:], in_=tri[:])
            ones33 = constp.tile([33, 64], BF16)
            nc.vector.memset(ones33[:], 0.0)
            nc.vector.memset(ones33[0:1, :], 1.0)
            nc.vector.memset(ones33[ds(32, 1), :], 1.0)

            wt = wtsp.tile([128, 8, QKV_COLS], BF16)       # [dchunk][768]
            for k in range(8):
                nc.sync.dma_start(out=wt[:, k, :], in_=wqkvT[ts(k, 128), :])
            wo = wtsp.tile([128, 8, D], BF16)
            for k in range(8):
                nc.sync.dma_start(out=wo[:, k, :], in_=woT[ts(k, 128), :])

            # persistent per-core tensors
            qt = seqp.tile([128, 2, S], BF16)   # Q^T  [pair, dk(2x64), q]
            kt = seqp.tile([128, 2, S], BF16)   # K^T
            vv = seqp.tile([128, NSC, HPC * (DK + 1)], BF16)   # V + ones col
            vv4 = vv[:].rearrange("p c (h e) -> p c h e", h=HPC)
            nc.vector.memset(vv4[:, :, :, DK:DK + 1], 1.0)
            outv = attp.tile([DK + 1, HPC, S], F32)  # vals + sums row
            attT = attp.tile([128, 2, S], BF16)
            rwo = attp.tile([128, 8, QSLICE], BF16)
            wsel_sb = attp.tile([128, 4], I32)
            sr2 = attp.tile([33, S], BF16)   # recip hi (row 0) / lo (row 32)
            nc.vector.memset(sr2[:, :], 0.0)

            # ------------- projection + RoPE + transposes (scoped x) -------
            with tc.tile_pool(name="xap", bufs=1) as xap:
                xa = xap.tile([128, 8, S], BF16)           # resident x^T
                for k in range(8):
                    nc.sync.dma_start(out=xa[:, k, :], in_=xT[ts(k, 128), :])
                cs = xap.tile([128, NSC, 2 * DK], F32)     # cos/sin gather
                for c in range(NSC):
                    pidx = xtp.tile([128, 1], I32, tag="pidx")
                    nc.sync.dma_start(out=pidx[:], in_=pos[ts(c, 128), :])
                    nc.gpsimd.indirect_dma_start(
                        out=cs[:, c, :],
                        out_offset=None,
                        in_=cstab[:],
                        in_offset=bass.IndirectOffsetOnAxis(
                            ap=pidx[:, 0:1], axis=0),
                    )

                for sc in range(NSC):
                    ps = pbig.tile([128, QKV_COLS], F32, space="PSUM",
                                   tag="big")
                    for k in range(8):
                        nc.tensor.matmul(
                            ps[:, 0:512], lhsT=xa[:, k, ts(sc, 128)],
                            rhs=wt[:, k, 0:512],
                            start=(k == 0), stop=(k == 7),
                        )
                    for k in range(8):
                        nc.tensor.matmul(
                            ps[:, 512:768], lhsT=xa[:, k, ts(sc, 128)],
                            rhs=wt[:, k, 512:768],
                            start=(k == 0), stop=(k == 7),
                        )

                    # RoPE over the Q,K halves (cols 0:512), 8 blocks of 64
                    def qk_ap(off):
                        a = ps[:]
                        return bass.AP(a.tensor, a.offset + off,
                                       [a.ap[0], [DK, 2 * HPC], [2, DK // 2]])

                    def cs_ap(off):
                        a = cs[:, sc, :]
                        return bass.AP(a.tensor, a.offset + off,
                                       [a.ap[0], [0, 2 * HPC], [2, DK // 2]])

                    t1 = ropet.tile([128, 2 * HPC, DK // 2], F32, tag="t1")
                    t2 = ropet.tile([128, 2 * HPC, DK // 2], F32, tag="t2")
                    t3 = ropet.tile([128, 2 * HPC, DK // 2], F32, tag="t3")
                    t4 = ropet.tile([128, 2 * HPC, DK // 2], F32, tag="t4")
                    roped = ropet.tile([128, QK_COLS], BF16, tag="roped")

                    def roped_ap(off):
                        a = roped[:]
                        return bass.AP(a.tensor, a.offset + off,
                                       [a.ap[0], [DK, 2 * HPC], [2, DK // 2]])

                    nc.vector.tensor_tensor(t1[:], qk_ap(0), cs_ap(0), MUL)
                    nc.vector.tensor_tensor(t2[:], qk_ap(1), cs_ap(DK), MUL)
                    nc.vector.tensor_tensor(roped_ap(0), t1[:], t2[:], SUB)
                    nc.vector.tensor_tensor(t3[:], qk_ap(0), cs_ap(DK), MUL)
                    nc.vector.tensor_tensor(t4[:], qk_ap(1), cs_ap(0), MUL)
                    nc.vector.tensor_tensor(roped_ap(1), t3[:], t4[:], ADD)

                    # V columns (ones col already set)
                    nc.vector.tensor_copy(
                        vv4[:, sc, :, 0:DK],
                        ps[:, 512:768].rearrange("p (h e) -> p h e", h=HPC),
                    )

                    # transpose the 4 roped q/k 128-col blocks -> qt/kt
                    for t in range(4):
                        tp = psmall.tile([128, 128], BF16, space="PSUM",
                                         tag="small")
                        nc.tensor.transpose(tp[:], roped[:, ts(t, 128)],
                                            ident[:])
                        dst = qt if t < 2 else kt
                        nc.vector.tensor_copy(dst[:, t % 2, ts(sc, 128)], tp[:])

            # ------------- attention (per head) + split AllGather ----------
            agin = [dramp.tile([4 * 128, QSLICE], BF16, name=f"agin{p}")
                    for p in range(2)]
            agout = [dramp.tile([4 * 512, QSLICE], BF16, name=f"agout{p}")
                     for p in range(2)]

            with tc.tile_pool(name="post", bufs=1) as postp:
                def emit_scores(h, kg, expp):
                    pr, hf = h // 2, (h % 2) * DK
                    etiles = []
                    for j in range(kg * KGRP, (kg + 1) * KGRP):
                        et = expp.tile([128, S], BF16, tag="exp")
                        # scores^T for k-chunk j: [128 k, q], q >= 128*j
                        for qh in range(2):
                            q0 = max(128 * j, 1024 * qh)
                            q1 = 1024 * (qh + 1)
                            if q0 >= q1:
                                continue
                            sp = pbig.tile([128, 1024], F32, space="PSUM",
                                           tag="big")
                            for qq in (1024 * qh, 1024 * qh + 512):
                                a, bnd = max(q0, qq), min(q1, qq + 512)
                                if a >= bnd:
                                    continue
                                nc.tensor.matmul(
                                    sp[:, ds(a - 1024 * qh, bnd - a)],
                                    lhsT=kt[ds(hf, DK), pr, ts(j, 128)],
                                    rhs=qt[ds(hf, DK), pr, ds(a, bnd - a)],
                                    start=True, stop=True,
                                )
                            nc.scalar.activation(
                                et[:, ds(q0, q1 - q0)],
                                sp[:, ds(q0 - 1024 * qh, q1 - q0)],
                                mybir.ActivationFunctionType.Exp,
                            )
                        # mask the diagonal block (q < k -> 0)
                        nc.vector.tensor_tensor(
                            et[:, ts(j, 128)], et[:, ts(j, 128)], tri_t[:],
                            MUL)
                        etiles.append((j, et))
                    return etiles

                def emit_vmms(h, kg, etiles):
                    for qc in range(kg, NQC):
                        part = psmall.tile([DK + 1, 512], F32, space="PSUM",
                                           tag="small")
                        for idx, (j, et) in enumerate(etiles):
                            a = max(512 * qc, 128 * j)
                            nc.tensor.matmul(
                                part[:, ds(a - 512 * qc, 512 * (qc + 1) - a)],
                                lhsT=vv[:, j, ds(h * (DK + 1), DK + 1)],
                                rhs=et[:, ds(a, 512 * (qc + 1) - a)],
                                start=(idx == 0), stop=(idx == KGRP - 1),
                            )
                        if kg == 0:
                            nc.vector.tensor_copy(
                                outv[:, h, ts(qc, 512)], part[:])
                        else:
                            nc.vector.tensor_tensor(
                                outv[:, h, ts(qc, 512)],
                                outv[:, h, ts(qc, 512)], part[:], ADD)

                def emit_normalize(h):
                    pr, hf = h // 2, (h % 2) * DK
                    srow = constp.tile([1, S], F32, tag="srow")
                    nc.vector.tensor_copy(srow[:], outv[ds(DK, 1), h, :])
                    nc.vector.reciprocal_approx_fast(srow[:], srow[:])
                    nc.vector.tensor_copy(sr2[0:1, :], srow[:])
                    nc.vector.tensor_tensor(
                        sr2[ds(32, 1), :], srow[:], sr2[0:1, :], SUB)
                    for qc in range(NQC):
                        rb = psmall.tile([DK, 512], F32, space="PSUM",
                                         tag="small")
                        nc.tensor.matmul(rb[:], lhsT=ones33[:],
                                         rhs=sr2[:, ts(qc, 512)],
                                         start=True, stop=True)
                        nc.vector.tensor_tensor(
                            attT[ds(hf, DK), pr, ts(qc, 512)],
                            outv[0:DK, h, ts(qc, 512)], rb[:], MUL)
                    if h % 2 == 1:     # pair complete -> stage + AllGather
                        p = h // 2
                        nc.sync.dma_start(
                            out=agin[p][:].rearrange("(d j) q -> d j q",
                                                     j=GROUPS),
                            in_=attT[:, p, :].rearrange("p (j q) -> p j q",
                                                        j=GROUPS),
                        )
                        nc.gpsimd.collective_compute(
                            "AllGather",
                            mybir.AluOpType.bypass,
                            ins=[agin[p][:]],
                            outs=[agout[p][:]],
                            replica_groups=[[0, 1, 2, 3], [4, 5, 6, 7]],
                        )

                with tc.tile_pool(name="expp", bufs=2 * KGRP + 1) as expp:
                    pending = None
                    for h in range(HPC):
                        for kg in range(NKG):
                            cur = emit_scores(h, kg, expp)
                            if pending is not None:
                                ph, pkg, pet = pending
                                emit_vmms(ph, pkg, pet)
                                if pkg == NKG - 1:
                                    emit_normalize(ph)
                            pending = (h, kg, cur)
                    ph, pkg, pet = pending
                    emit_vmms(ph, pkg, pet)
                    emit_normalize(ph)

                # ------------- output projection (q-slice), two passes -----
                for c in range(4):
                    nc.sync.dma_start(
                        out=wsel_sb[:, c:c + 1], in_=wsel[ts(c, 128), :])
                for dp in (0, 2, 4, 6, 1, 3, 5, 7):
                    nc.gpsimd.indirect_dma_start(
                        out=rwo[:, dp, :],
                        out_offset=None,
                        in_=agout[dp % 2][:],
                        in_offset=bass.IndirectOffsetOnAxis(
                            ap=wsel_sb[:, dp // 2:dp // 2 + 1], axis=0),
                    )
                fe = postp.tile([128, 8, QSLICE], F32)
                for ec in range(8):
                    fp = pbig.tile([128, QSLICE], F32, space="PSUM", tag="big")
                    for i, dp in enumerate((0, 2, 4, 6)):
                        nc.tensor.matmul(
                            fp[:], lhsT=wo[:, dp, ts(ec, 128)],
                            rhs=rwo[:, dp, :],
                            start=(i == 0), stop=(i == 3),
                        )
                    nc.vector.tensor_copy(fe[:, ec, :], fp[:])
                for ec in range(8):
                    fp = pbig.tile([128, QSLICE], F32, space="PSUM", tag="big")
                    for i, dp in enumerate((1, 3, 5, 7)):
                        nc.tensor.matmul(
                            fp[:], lhsT=wo[:, dp, ts(ec, 128)],
                            rhs=rwo[:, dp, :],
                            start=(i == 0), stop=(i == 3),
                        )
                    fin_sb = xtp.tile([128, QSLICE], F32, tag="fin")
                    nc.vector.tensor_tensor(fin_sb[:], fe[:, ec, :], fp[:],
                                            ADD)
                    nc.sync.dma_start(out=finT[ts(ec, 128), :], in_=fin_sb[:])

    nc.compile()
    return nc


def _host_prep(x, token_positions, W_qkv, W_o):
    bf16 = ml_dtypes.bfloat16
    xT = np.ascontiguousarray(np.transpose(x, (0, 2, 1))).astype(bf16)  # [B,D,S]

    # per-group W_qkv^T slices (Q rows pre-scaled by 1/sqrt(dk))
    wq = W_qkv[0 * D:1 * D] * np.float32(1.0 / np.sqrt(DK))
    wk = W_qkv[1 * D:2 * D]
    wv = W_qkv[2 * D:3 * D]
    wslices = []
    for g in range(GROUPS):
        rows = slice(g * HPC * DK, (g + 1) * HPC * DK)
        wsl = np.concatenate([wq[rows], wk[rows], wv[rows]], axis=0)  # [768, D]
        wslices.append(np.ascontiguousarray(wsl.T).astype(bf16))      # [D, 768]

    woT = np.ascontiguousarray(W_o.T).astype(bf16)                    # [D, D]

    idx = np.arange(DK // 2, dtype=np.float64)
    freqs = 1.0 / (THETA ** (2.0 * idx / DK))
    ang = np.arange(MAXPOS, dtype=np.float64)[:, None] * freqs[None, :]
    cstab = np.zeros((MAXPOS, 2 * DK), dtype=np.float32)
    cstab[:, 0:DK:2] = np.cos(ang)
    cstab[:, 1:DK:2] = np.cos(ang)
    cstab[:, DK::2] = np.sin(ang)
    cstab[:, DK + 1::2] = np.sin(ang)

    tri = (np.arange(128)[None, :] >= np.arange(128)[:, None]).astype(bf16)

    posi = np.asarray(token_positions).astype(np.int32).reshape(B, S, 1)

    # wsel[c*128 + r] selects row 512*c + 4*r + g of agout[dp % 2] for
    # d-chunk dp = 2*c + (0 or 1); same values for both halves.
    rr = np.arange(QSLICE)
    in_maps = []
    for c in range(N_CORES):
        b, g = c // GROUPS, c % GROUPS
        wsel = 512 * (rr // 128) + 4 * (rr % 128) + g
        in_maps.append({
            "xT": np.asarray(xT[b]),
            "wqkvT": wslices[g],
            "woT": woT,
            "cstab": cstab,
            "pos": np.ascontiguousarray(posi[b]),
            "tri": tri,
            "wsel": wsel.astype(np.int32).reshape(QSLICE, 1),
        })
    return in_maps


def _assemble(results):
    out = np.empty((B, S, D), dtype=np.float32)
    for b in range(B):
        fullT = np.concatenate(
            [results[b * GROUPS + g]["finT"] for g in range(GROUPS)], axis=1)
        out[b] = fullT.T
    return out


_NC_CACHE = {}


def run(inputs, trace=False, **kw):
    if "nc" not in _NC_CACHE:
        _NC_CACHE["nc"] = _build()
    nc = _NC_CACHE["nc"]
    in_maps = _host_prep(**inputs)
    res = run_bass_kernel_spmd(
        nc, in_maps, core_ids=list(range(N_CORES)), trace=trace, **kw)
    return _assemble(res.results), res


def kernel(**inputs):
    out, _ = run(inputs, trace=False)
    return out


# revision 25
# speedup vs baseline: 1.3848x; 1.1038x over previous
"""Multi-head self-attention (RoPE, causal) on 8 trn2 NeuronCores.

Sharding: core c -> batch b = c // 4, head group g = c % 4 (4 heads each).
Each core:
  - projects Q,K,V for its batch / its 4 heads (token-major), applies RoPE
    via a gathered cos/sin table, transposes Q,K to [dk, S] layout,
  - computes scores^T = K^T-chunk x Q (k on partitions), exp on ACT,
    then out^T[d, q] accumulated via V'-stationary matmuls where V' carries
    an appended ones-column so the softmax sums ride along as row 64;
    score groups are emitted one group ahead of the V-matmuls so the PE
    keeps the ACT exp stream fed,
  - normalizes with a fast reciprocal + bf16 hi/lo ones-broadcast matmuls,
  - AllGathers (4-core batch group, split per head-pair to overlap with
    compute) the head-sharded attention output, packed so a per-core index
    gather (wsel input) extracts this core's 512-token q-slice with an
    SPMD-uniform program; the output projection consumes the first AG's
    half early (two-pass accumulation) to shorten the serial tail.
Host only reshapes/transposes/casts inputs and concatenates outputs.
"""

import sys

for _p in ("/opt/trn_rl_repo",):
    if _p not in sys.path:
        sys.path.append(_p)

import numpy as np
import ml_dtypes

import concourse.bass as bass
import concourse.mybir as mybir
import concourse.tile as tile
from concourse import bacc
from concourse.bass import ds, ts
from concourse.bass_utils import run_bass_kernel_spmd
from concourse.masks import make_identity

BF16 = mybir.dt.bfloat16
F32 = mybir.dt.float32
I32 = mybir.dt.int32

B, S, D = 2, 2048, 1024
H, DK = 16, 64
THETA = 10000.0
MAXPOS = 2048
N_CORES = 8
GROUPS = 4          # head groups (cores) per batch
HPC = H // GROUPS   # heads per core = 4
QKV_COLS = 3 * HPC * DK        # 768 per-core projection width
QK_COLS = 2 * HPC * DK         # 512 (Q then K)
NSC = S // 128                 # 16 token chunks
NQC = S // 512                 # 4 q column-chunks
QSLICE = S // GROUPS           # 512 output tokens per core
KGRP = 2                       # k-chunks per exp/V-matmul group
NKG = NSC // KGRP
MUL = mybir.AluOpType.mult
ADD = mybir.AluOpType.add
SUB = mybir.AluOpType.subtract


def _build():
    nc = bacc.Bacc("TRN2", num_devices=N_CORES)

    xT = nc.dram_tensor("xT", [D, S], BF16, kind="ExternalInput")
    wqkvT = nc.dram_tensor("wqkvT", [D, QKV_COLS], BF16, kind="ExternalInput")
    woT = nc.dram_tensor("woT", [D, D], BF16, kind="ExternalInput")
    cstab = nc.dram_tensor("cstab", [MAXPOS, 2 * DK], F32, kind="ExternalInput")
    pos = nc.dram_tensor("pos", [S, 1], I32, kind="ExternalInput")
    tri = nc.dram_tensor("tri", [128, 128], BF16, kind="ExternalInput")
    wsel = nc.dram_tensor("wsel", [QSLICE, 1], I32, kind="ExternalInput")
    finT = nc.dram_tensor("finT", [D, QSLICE], F32, kind="ExternalOutput")

    with tile.TileContext(nc) as tc:
        with (
            tc.tile_pool(name="const", bufs=1) as constp,
            tc.tile_pool(name="wts", bufs=1) as wtsp,
            tc.tile_pool(name="seq", bufs=1) as seqp,
            tc.tile_pool(name="xtp", bufs=3) as xtp,
            tc.tile_pool(name="ropet", bufs=2) as ropet,
            tc.tile_pool(name="attp", bufs=1) as attp,
            tc.tile_pool(name="pbig", bufs=3, space="PSUM") as pbig,
            tc.tile_pool(name="psmall", bufs=2, space="PSUM") as psmall,
            tc.tile_pool(name="dram", bufs=1, space="DRAM") as dramp,
        ):
            # ---------------- constants + resident weights ----------------
            ident = constp.tile([128, 128], BF16)
            make_identity(nc, ident[:])
            tri_t = constp.tile([128, 128], BF16)
            nc.sync.dma_start(out=tri_t[:], in_=tri[:])
            ones33 = constp.tile([33, 64], BF16)
            nc.vector.memset(ones33[:], 0.0)
            nc.vector.memset(ones33[0:1, :], 1.0)
            nc.vector.memset(ones33[ds(32, 1), :], 1.0)

            wt = wtsp.tile([128, 8, QKV_COLS], BF16)       # [dchunk][768]
            for k in range(8):
                nc.sync.dma_start(out=wt[:, k, :], in_=wqkvT[ts(k, 128), :])
            wo = wtsp.tile([128, 8, D], BF16)
            for k in range(8):
                nc.sync.dma_start(out=wo[:, k, :], in_=woT[ts(k, 128), :])

            # persistent per-core tensors
            qt = seqp.tile([128, 2, S], BF16)   # Q^T  [pair, dk(2x64), q]
            kt = seqp.tile([128, 2, S], BF16)   # K^T
            vv = seqp.tile([128, NSC, HPC * (DK + 1)], BF16)   # V + ones col
            vv4 = vv[:].rearrange("p c (h e) -> p c h e", h=HPC)
            nc.vector.memset(vv4[:, :, :, DK:DK + 1], 1.0)
            outv = attp.tile([DK + 1, HPC, S], F32)  # vals + sums row
            attT = attp.tile([128, 2, S], BF16)
            rwo = attp.tile([128, 8, QSLICE], BF16)
            wsel_sb = attp.tile([128, 4], I32)
            sr2 = attp.tile([33, S], BF16)   # recip hi (row 0) / lo (row 32)
            nc.vector.memset(sr2[:, :], 0.0)

            # ------------- projection + RoPE + transposes (scoped x) -------
            with tc.tile_pool(name="xap", bufs=1) as xap:
                xa = xap.tile([128, 8, S], BF16)           # resident x^T
                for k in range(8):
                    nc.sync.dma_start(out=xa[:, k, :], in_=xT[ts(k, 128), :])
                cs = xap.tile([128, NSC, 2 * DK], F32)     # cos/sin gather
                for c in range(NSC):
                    pidx = xtp.tile([128, 1], I32, tag="pidx")
                    nc.sync.dma_start(out=pidx[:], in_=pos[ts(c, 128), :])
                    nc.gpsimd.indirect_dma_start(
                        out=cs[:, c, :],
                        out_offset=None,
                        in_=cstab[:],
                        in_offset=bass.IndirectOffsetOnAxis(
                            ap=pidx[:, 0:1], axis=0),
                    )

                for sc in range(NSC):
                    ps = pbig.tile([128, QKV_COLS], F32, space="PSUM",
                                   tag="big")
                    for k in range(8):
                        nc.tensor.matmul(
                            ps[:, 0:512], lhsT=xa[:, k, ts(sc, 128)],
                            rhs=wt[:, k, 0:512],
                            start=(k == 0), stop=(k == 7),
                        )
                    for k in range(8):
                        nc.tensor.matmul(
                            ps[:, 512:768], lhsT=xa[:, k, ts(sc, 128)],
                            rhs=wt[:, k, 512:768],
                            start=(k == 0), stop=(k == 7),
                        )

                    # RoPE over the Q,K halves (cols 0:512), 8 blocks of 64
                    def qk_ap(off):
                        a = ps[:]
                        return bass.AP(a.tensor, a.offset + off,
                                       [a.ap[0], [DK, 2 * HPC], [2, DK // 2]])

                    def cs_ap(off):
                        a = cs[:, sc, :]
                        return bass.AP(a.tensor, a.offset + off,
                                       [a.ap[0], [0, 2 * HPC], [2, DK // 2]])

                    t1 = ropet.tile([128, 2 * HPC, DK // 2], F32, tag="t1")
                    t2 = ropet.tile([128, 2 * HPC, DK // 2], F32, tag="t2")
                    t3 = ropet.tile([128, 2 * HPC, DK // 2], F32, tag="t3")
                    t4 = ropet.tile([128, 2 * HPC, DK // 2], F32, tag="t4")
                    roped = ropet.tile([128, QK_COLS], BF16, tag="roped")

                    def roped_ap(off):
                        a = roped[:]
                        return bass.AP(a.tensor, a.offset + off,
                                       [a.ap[0], [DK, 2 * HPC], [2, DK // 2]])

                    nc.vector.tensor_tensor(t1[:], qk_ap(0), cs_ap(0), MUL)
                    nc.vector.tensor_tensor(t2[:], qk_ap(1), cs_ap(DK), MUL)
                    nc.vector.tensor_tensor(roped_ap(0), t1[:], t2[:], SUB)
                    nc.vector.tensor_tensor(t3[:], qk_ap(0), cs_ap(DK), MUL)
                    nc.vector.tensor_tensor(t4[:], qk_ap(1), cs_ap(0), MUL)
                    nc.vector.tensor_tensor(roped_ap(1), t3[:], t4[:], ADD)

                    # V columns (ones col already set)
                    nc.vector.tensor_copy(
                        vv4[:, sc, :, 0:DK],
                        ps[:, 512:768].rearrange("p (h e) -> p h e", h=HPC),
                    )

                    # transpose the 4 roped q/k 128-col blocks -> qt/kt
                    for t in range(4):
                        tp = psmall.tile([128, 128], BF16, space="PSUM",
                                         tag="small")
                        nc.tensor.transpose(tp[:], roped[:, ts(t, 128)],
                                            ident[:])
                        dst = qt if t < 2 else kt
                        nc.vector.tensor_copy(dst[:, t % 2, ts(sc, 128)], tp[:])

            # ------------- attention (per head) + split AllGather ----------
            agin = [dramp.tile([4 * 128, QSLICE], BF16, name=f"agin{p}")
                    for p in range(2)]
            agout = [dramp.tile([4 * 512, QSLICE], BF16, name=f"agout{p}")
                     for p in range(2)]

            with tc.tile_pool(name="post", bufs=1) as postp:
                def emit_scores(h, kg, expp):
                    pr, hf = h // 2, (h % 2) * DK
                    etiles = []
                    for j in range(kg * KGRP, (kg + 1) * KGRP):
                        et = expp.tile([128, S], BF16, tag="exp")
                        # scores^T for k-chunk j: [128 k, q], q >= 128*j
                        for qh in range(2):
                            q0 = max(128 * j, 1024 * qh)
                            q1 = 1024 * (qh + 1)
                            if q0 >= q1:
                                continue
                            sp = pbig.tile([128, 1024], F32, space="PSUM",
                                           tag="big")
                            for qq in (1024 * qh, 1024 * qh + 512):
                                a, bnd = max(q0, qq), min(q1, qq + 512)
                                if a >= bnd:
                                    continue
                                nc.tensor.matmul(
                                    sp[:, ds(a - 1024 * qh, bnd - a)],
                                    lhsT=kt[ds(hf, DK), pr, ts(j, 128)],
                                    rhs=qt[ds(hf, DK), pr, ds(a, bnd - a)],
                                    start=True, stop=True,
                                )
                            nc.scalar.activation(
                                et[:, ds(q0, q1 - q0)],
                                sp[:, ds(q0 - 1024 * qh, q1 - q0)],
                                mybir.ActivationFunctionType.Exp,
                            )
                        # mask the diagonal block (q < k -> 0)
                        nc.vector.tensor_tensor(
                            et[:, ts(j, 128)], et[:, ts(j, 128)], tri_t[:],
                            MUL)
                        etiles.append((j, et))
                    return etiles

                def emit_vmms(h, kg, etiles):
                    for qc in range(kg, NQC):
                        part = psmall.tile([DK + 1, 512], F32, space="PSUM",
                                           tag="small")
                        for idx, (j, et) in enumerate(etiles):
                            a = max(512 * qc, 128 * j)
                            nc.tensor.matmul(
                                part[:, ds(a - 512 * qc, 512 * (qc + 1) - a)],
                                lhsT=vv[:, j, ds(h * (DK + 1), DK + 1)],
                                rhs=et[:, ds(a, 512 * (qc + 1) - a)],
                                start=(idx == 0), stop=(idx == KGRP - 1),
                            )
                        if kg == 0:
                            nc.vector.tensor_copy(
                                outv[:, h, ts(qc, 512)], part[:])
                        else:
                            nc.vector.tensor_tensor(
                                outv[:, h, ts(qc, 512)],
                                outv[:, h, ts(qc, 512)], part[:], ADD)

                def emit_normalize(h):
                    pr, hf = h // 2, (h % 2) * DK
                    srow = constp.tile([1, S], F32, tag="srow")
                    nc.vector.tensor_copy(srow[:], outv[ds(DK, 1), h, :])
                    nc.vector.reciprocal_approx_fast(srow[:], srow[:])
                    nc.vector.tensor_copy(sr2[0:1, :], srow[:])
                    nc.vector.tensor_tensor(
                        sr2[ds(32, 1), :], srow[:], sr2[0:1, :], SUB)
                    for qc in range(NQC):
                        rb = psmall.tile([DK, 512], F32, space="PSUM",
                                         tag="small")
                        nc.tensor.matmul(rb[:], lhsT=ones33[:],
                                         rhs=sr2[:, ts(qc, 512)],
                                         start=True, stop=True)
                        nc.vector.tensor_tensor(
                            attT[ds(hf, DK), pr, ts(qc, 512)],
                            outv[0:DK, h, ts(qc, 512)], rb[:], MUL)
                    if h % 2 == 1:     # pair complete -> stage + AllGather
                        p = h // 2
                        nc.sync.dma_start(
                            out=agin[p][:].rearrange("(d j) q -> d j q",
                                                     j=GROUPS),
                            in_=attT[:, p, :].rearrange("p (j q) -> p j q",
                                                        j=GROUPS),
                        )
                        nc.gpsimd.collective_compute(
                            "AllGather",
                            mybir.AluOpType.bypass,
                            ins=[agin[p][:]],
                            outs=[agout[p][:]],
                            replica_groups=[[0, 1, 2, 3], [4, 5, 6, 7]],
                        )

                with tc.tile_pool(name="expp", bufs=2 * KGRP + 1) as expp:
                    pending = None
                    for h in range(HPC):
                        for kg in range(NKG):
                            cur = emit_scores(h, kg, expp)
                            if pending is not None:
                                ph, pkg, pet = pending
                                emit_vmms(ph, pkg, pet)
                                if pkg == NKG - 1:
                                    emit_normalize(ph)
                            pending = (h, kg, cur)
                    ph, pkg, pet = pending
                    emit_vmms(ph, pkg, pet)
                    emit_normalize(ph)

                # ------------- output projection (q-slice), two passes -----
                for c in range(4):
                    nc.sync.dma_start(
                        out=wsel_sb[:, c:c + 1], in_=wsel[ts(c, 128), :])
                for dp in (0, 2, 4, 6, 1, 3, 5, 7):
                    nc.gpsimd.indirect_dma_start(
                        out=rwo[:, dp, :],
                        out_offset=None,
                        in_=agout[dp % 2][:],
                        in_offset=bass.IndirectOffsetOnAxis(
                            ap=wsel_sb[:, dp // 2:dp // 2 + 1], axis=0),
                    )
                fe = postp.tile([128, 8, QSLICE], F32)
                for ec in range(8):
                    fp = pbig.tile([128, QSLICE], F32, space="PSUM", tag="big")
                    for i, dp in enumerate((0, 2, 4, 6)):
                        nc.tensor.matmul(
                            fp[:], lhsT=wo[:, dp, ts(ec, 128)],
                            rhs=rwo[:, dp, :],
                            start=(i == 0), stop=(i == 3),
                        )
                    nc.vector.tensor_copy(fe[:, ec, :], fp[:])
                for ec in range(8):
                    fp = pbig.tile([128, QSLICE], F32, space="PSUM", tag="big")
                    for i, dp in enumerate((1, 3, 5, 7)):
                        nc.tensor.matmul(
                            fp[:], lhsT=wo[:, dp, ts(ec, 128)],
                            rhs=rwo[:, dp, :],
                            start=(i == 0), stop=(i == 3),
                        )
                    fin_sb = xtp.tile([128, QSLICE], F32, tag="fin")
                    nc.vector.tensor_tensor(fin_sb[:], fe[:, ec, :], fp[:],
                                            ADD)
                    nc.sync.dma_start(out=finT[ts(ec, 128), :], in_=fin_sb[:])

    nc.compile()
    return nc


def _host_prep(x, token_positions, W_qkv, W_o):
    bf16 = ml_dtypes.bfloat16
    xT = np.ascontiguousarray(np.transpose(x, (0, 2, 1))).astype(bf16)  # [B,D,S]

    # per-group W_qkv^T slices (Q rows pre-scaled by 1/sqrt(dk))
    wq = W_qkv[0 * D:1 * D] * np.float32(1.0 / np.sqrt(DK))
    wk = W_qkv[1 * D:2 * D]
    wv = W_qkv[2 * D:3 * D]
    wslices = []
    for g in range(GROUPS):
        rows = slice(g * HPC * DK, (g + 1) * HPC * DK)
        wsl = np.concatenate([wq[rows], wk[rows], wv[rows]], axis=0)  # [768, D]
        wslices.append(np.ascontiguousarray(wsl.T).astype(bf16))      # [D, 768]

    woT = np.ascontiguousarray(W_o.T).astype(bf16)                    # [D, D]

    idx = np.arange(DK // 2, dtype=np.float64)
    freqs = 1.0 / (THETA ** (2.0 * idx / DK))
    ang = np.arange(MAXPOS, dtype=np.float64)[:, None] * freqs[None, :]
    cstab = np.zeros((MAXPOS, 2 * DK), dtype=np.float32)
    cstab[:, 0:DK:2] = np.cos(ang)
    cstab[:, 1:DK:2] = np.cos(ang)
    cstab[:, DK::2] = np.sin(ang)
    cstab[:, DK + 1::2] = np.sin(ang)

    tri = (np.arange(128)[None, :] >= np.arange(128)[:, None]).astype(bf16)

    posi = np.asarray(token_positions).astype(np.int32).reshape(B, S, 1)

    # wsel[c*128 + r] selects row 512*c + 4*r + g of agout[dp % 2] for
    # d-chunk dp = 2*c + (0 or 1); same values for both halves.
    rr = np.arange(QSLICE)
    in_maps = []
    for c in range(N_CORES):
        b, g = c // GROUPS, c % GROUPS
        wsel = 512 * (rr // 128) + 4 * (rr % 128) + g
        in_maps.append({
            "xT": np.asarray(xT[b]),
            "wqkvT": wslices[g],
            "woT": woT,
            "cstab": cstab,
            "pos": np.ascontiguousarray(posi[b]),
            "tri": tri,
            "wsel": wsel.astype(np.int32).reshape(QSLICE, 1),
        })
    return in_maps


def _assemble(results):
    out = np.empty((B, S, D), dtype=np.float32)
    for b in range(B):
        fullT = np.concatenate(
            [results[b * GROUPS + g]["finT"] for g in range(GROUPS)], axis=1)
        out[b] = fullT.T
    return out


_NC_CACHE = {}


def run(inputs, trace=False, **kw):
    if "nc" not in _NC_CACHE:
        _NC_CACHE["nc"] = _build()
    nc = _NC_CACHE["nc"]
    in_maps = _host_prep(**inputs)
    res = run_bass_kernel_spmd(
        nc, in_maps, core_ids=list(range(N_CORES)), trace=trace, **kw)
    return _assemble(res.results), res


def kernel(**inputs):
    out, _ = run(inputs, trace=False)
    return out
